# revision 31
# baseline (speedup 1.0000x reference)
"""EdgeNetwork GNN message-passing kernel for 8 Trainium2 NeuronCores.

Math (per batch b):
    bo = Ro[b]^T @ X[b]            # [E, F]  gather of outgoing-node feats
    bi = Ri[b]^T @ X[b]            # [E, F]
    feats = concat(bo, bi)         # [E, 2F]
    h = tanh(feats @ W1 + b1)      # [E, H]
    out = sigmoid(h @ W2 + b2)     # [E]

Sharding: 8 shards over (B=2) x (E/4): each core owns one (b, 2048-edge)
slice with zero cross-core communication.

Fast path: Ri/Ro produced by jax.nn.one_hot are exactly one-hot, so the
einsum is a row gather X[idx].  The host losslessly re-encodes each
one-hot matrix as its int16 index vector (and verifies exactness with
probe GEMVs -- any non-one-hot input falls back to the dense kernel).
On device an ap_gather over a replicated X^T table delivers the
gathered features already transposed (features-on-partitions), feeding
the MLP directly.

This revision vs the 43 us baseline:
  * X^T ships as bf16 [128, 4096] (1 MB, host-replicated) and is
    upcast on-chip to the f32 gather table split across DVE + Act,
    instead of DMAing the 2 MB f32 table (which took ~10 us and held
    the gather until ~19.7 us).
  * The single 512-idx ap_gather (14 us of Q7 time, the dominant cost:
    ~110 ns per 4-idx request, invariant to layout) is split into 4
    column-range chunks of 128 idxs; all 8 Q7 cores stay busy in each
    chunk and the MLP for chunk k runs under the gather of chunk k+1.
  * The MLP runs in bf16: one K=32 layer-1 matmul per chunk (the DVE
    rebase-casts double as the f32 -> bf16 convert), tanh -> bf16,
    K=64 layer-2, sigmoid into partitions 0/32/64/96, one output DMA.
    Host reorders the chunk-major output (pure index bookkeeping).

Dense fallback: streams the full one-hot matrices through the
TensorEngine as float32r matmuls (HBM-bound, 64 MB/core).
"""

import numpy as np
import ml_dtypes

import concourse.bass as bass
import concourse.mybir as mybir
import concourse.tile as tile
from concourse import library_config
from concourse.bass_utils import run_bass_kernel_spmd

B, N, E, F, H = 2, 4096, 8192, 16, 64
NCORES = 8
SPB = NCORES // B          # edge shards per batch = 4
EPC = E // SPB             # edges per core = 2048
P = 128                    # partition size
NCH = N // P               # 32 node chunks
CQ = 3                     # node chunks per big DMA transfer (dense path)
GROUPS = [(g * CQ, CQ) for g in range(10)] + [(30, 1), (31, 1)]
NB = 512                   # PSUM bank width in f32
NCB = EPC // NB            # 4 column blocks per core

_f32 = mybir.dt.float32
_f32r = mybir.dt.float32r
_bf16 = mybir.dt.bfloat16
_i16 = mybir.dt.int16
_AF = mybir.ActivationFunctionType
_BF = ml_dtypes.bfloat16

# ap_gather layout: 4 edge groups (slots) of 512, each owning a
# 32-partition slice: even Q7 core = 16 Ro-gather feature channels,
# odd Q7 core = 16 Ri-gather channels, so each slot holds the
# [bo; bi] feats concat and layer 1 is a single K=32 matmul.
GQ = 4
GN = EPC // GQ             # 512 idxs per Q7 core (= slot edge count)

# The gather is chunked along the idx dimension so the MLP pipelines
# under it (all 8 Q7 cores stay busy in every chunk).  Each ap_gather
# instruction costs ~1.3 us beyond its ~96 ns/4-idx request stream, so
# two chunks beat four (measured: 4x128 = 4.7 us each); the second
# chunk is smaller so the un-hidden tail MLP after the last gather is
# shorter, while chunk 1's larger MLP still fits under chunk 2's
# gather shadow.
CWS = [320, 192]           # idxs per core per chunk (sum = GN = 512)
COFF = [0, 320]
NCHUNK = len(CWS)
# 512-col half-splits of each chunk's 4*CW feat columns (PSUM banks).
CHALVES = [[(0, 512), (512, 512), (1024, 256)], [(0, 512), (512, 256)]]
NROWS = sum(len(h) for h in CHALVES)  # output rows (one per half)
# Activation outputs may only start at partitions 0/32/64/96, so the 5
# rows pack into 4 partition slots x 1024 columns: (partition slot, col).
RPLACE = [(0, 0), (1, 0), (2, 0), (3, 0), (0, NB)]
OUTW = 2 * NB
# The 1 MB bf16 table ships as two sequential column-half DMAs on the
# sync queue so half 1 upcasts under half 2's transfer.  Half 2's
# upcast is split between DVE (~0.69 ns/col) and Act (~1.28 ns/col).
XH = N // 2
UPC_DVE = XH + 1408
AUXW = 104                 # packed idx+weights tensor width (bf16 cols)


def _build_emap():
    """_EMAP_SRC[e] = flat index into the device's [4, OUTW] output
    holding edge e's sigmoid (e relative to the core's 2048-edge slice)."""
    src = np.empty(EPC, np.int64)
    row = 0
    for k in range(NCHUNK):
        cw = CWS[k]
        for s, w in CHALVES[k]:
            slot, col = RPLACE[row]
            c = s + np.arange(w)
            edge = 512 * (c // cw) + COFF[k] + (c % cw)
            src[edge] = slot * OUTW + col + np.arange(w)
            row += 1
    return src


for _k in range(NCHUNK):
    assert sum(w for _, w in CHALVES[_k]) == GQ * CWS[_k]
_EMAP_SRC = _build_emap()


# ---------------------------------------------------------------------------
# ap_gather device kernel: on-chip free-dim gather over X^T
# ---------------------------------------------------------------------------

def _build_nc_apg() -> bass.Bass:
    nc = bass.Bass()

    xtb = nc.dram_tensor("xtb", [P, N], _bf16, kind="ExternalInput")
    # idx + W1 + b1 + W2 + b2 packed into one raw-bytes tensor: every DMA
    # instruction costs ~0.65 us of sequencer issue time, so one DMA
    # replaces five.  Layout (bf16 columns): [0:32) idx int16, [32:96)
    # W1 bf16 on partitions 0-31, [96:98) b1 f32 on partitions 0-63,
    # [98:99) W2 bf16 on partitions 0-63, [100:102) b2 f32 on partition 0.
    aux = nc.dram_tensor("aux", [P, AUXW], _bf16, kind="ExternalInput")
    out = nc.dram_tensor("out", [4, OUTW], _f32, kind="ExternalOutput")

    with tile.TileContext(nc) as tc:
        with (
            tc.tile_pool(name="consts", bufs=1) as consts,
            tc.tile_pool(name="mid", bufs=1) as mid,
            tc.tile_pool(name="ps", bufs=8, space="PSUM") as ps,
        ):
            nc.gpsimd.load_library(library_config.ap_gather)

            # Table halves go back-to-back on the sync queue (its rings
            # process them FIFO, so half 1 lands ~2 us before half 2) as
            # SEPARATE tiles -- a split write into one tile picks up
            # framework waits that defeat the pre-boot hoist.  The packed
            # aux blob rides the scalar queue.
            xtb1_sb = mid.tile([P, XH], _bf16)
            nc.sync.dma_start(out=xtb1_sb, in_=xtb.ap()[:, 0:XH])
            xtb2_sb = mid.tile([P, N - XH], _bf16)
            nc.sync.dma_start(out=xtb2_sb, in_=xtb.ap()[:, XH:N])
            aux_sb = consts.tile([P, AUXW], _bf16)
            nc.scalar.dma_start(out=aux_sb, in_=aux.ap())

            idx_sb = aux_sb[:, 0:32]
            w1_sb = aux_sb[0 : 2 * F, 32:96]
            b1_sb = aux_sb[0:H, 96:98].bitcast(_f32)
            w2_sb = aux_sb[0:H, 98:99]
            b2_sb = aux_sb[0:1, 100:102].bitcast(_f32)

            # Preload the activation LUT set on the Act engine before its
            # upcast share (the first activation otherwise pays the ~1.3 us
            # table load on the critical path).
            warm = consts.tile([H, 1], _f32)
            nc.scalar.activation(warm[:], b1_sb, _AF.Tanh, bias=b1_sb)
            nc.scalar.activation(warm[:], b1_sb, _AF.Sigmoid, bias=b1_sb)

            # NB: the gather table must be plain f32 -- a float32r ISA dtype
            # on the ap_gather instruction wedges the Q7 (measured), and
            # bf16 is illegal for d=1 (elem word must be 4-byte-aligned).
            # DVE upcasts half 1 while half 2 is still in flight; half 2
            # is then split DVE/Act so both finish together.
            xt_sb = mid.tile([P, N, 1], _f32)
            nc.vector.tensor_copy(xt_sb[:, 0:XH, 0], xtb1_sb[:])
            nc.vector.tensor_copy(
                xt_sb[:, XH:UPC_DVE, 0], xtb2_sb[:, 0 : UPC_DVE - XH]
            )
            nc.scalar.copy(xt_sb[:, UPC_DVE:N, 0], xtb2_sb[:, UPC_DVE - XH :])

            dsts = []
            for k in range(NCHUNK):
                dst = mid.tile([P, CWS[k], 1], _f32, name=f"dst_{k}")
                nc.gpsimd.ap_gather(
                    out_ap=dst[:],
                    in_ap=xt_sb[:],
                    idxs_ap=idx_sb[
                        :, COFF[k] // 16 : (COFF[k] + CWS[k]) // 16
                    ].bitcast(_i16),
                    channels=P,
                    num_elems=N,
                    d=1,
                    num_idxs=CWS[k],
                )
                dsts.append(dst)

            out_sb = mid.tile([P, OUTW], _f32)
            row = 0
            for k in range(NCHUNK):
                cw = CWS[k]
                # Rebase each 32-partition slot (= [bo; bi] feats block) to
                # partition 0 for the PE while converting f32 -> bf16.
                fc = mid.tile([2 * F, GQ * cw], _bf16, name=f"fc_{k}")
                for q in range(GQ):
                    nc.vector.tensor_copy(
                        fc[:, q * cw : (q + 1) * cw],
                        dsts[k][32 * q : 32 * q + 2 * F, :, 0],
                    )
                fcf = fc[:]
                # The chunk's 4*cw columns run as <=512-col slices (one PSUM
                # bank each).  Issue stage-by-stage across the slices so the
                # PE never idles waiting for a tanh of the slice it just
                # produced.
                halves = CHALVES[k]
                hps, hsbs, lps = [], [], []
                for i, (s, w) in enumerate(halves):
                    hp = ps.tile([H, w], _f32, tag="bank", name=f"hp_{row+i}")
                    nc.tensor.matmul(
                        hp[:], w1_sb, fcf[:, s : s + w], start=True, stop=True
                    )
                    hps.append(hp)
                for i, (s, w) in enumerate(halves):
                    h_sb = mid.tile([H, w], _bf16, name=f"h_{row+i}")
                    nc.scalar.activation(
                        h_sb[:], hps[i][:], _AF.Tanh, bias=b1_sb
                    )
                    hsbs.append(h_sb)
                for i, (s, w) in enumerate(halves):
                    lp = ps.tile([1, w], _f32, tag="bank", name=f"lp_{row+i}")
                    nc.tensor.matmul(
                        lp[:], w2_sb, hsbs[i][:], start=True, stop=True
                    )
                    lps.append(lp)
                for i, (s, w) in enumerate(halves):
                    # Output rows pack into partitions 0/32/64/96 (the only
                    # legal Activation output bases) x 1024 cols, so the
                    # final DMA is one multi-partition transfer.
                    slot, col = RPLACE[row + i]
                    nc.scalar.activation(
                        out_sb[32 * slot : 32 * slot + 1, col : col + w],
                        lps[i][:],
                        _AF.Sigmoid,
                        bias=b2_sb,
                    )
                row += len(halves)
            nc.sync.dma_start(out=out.ap(), in_=out_sb[0:P:32, :])

    return nc


# ---------------------------------------------------------------------------
# Dense fallback device kernel (streams full one-hot matrices)
# ---------------------------------------------------------------------------

def _build_nc_dense() -> bass.Bass:
    nc = bass.Bass()

    x = nc.dram_tensor("x", [P, NCH * F], _f32r, kind="ExternalInput")
    ro = nc.dram_tensor("ro", [N, EPC], _f32r, kind="ExternalInput")
    ri = nc.dram_tensor("ri", [N, EPC], _f32r, kind="ExternalInput")
    w1 = nc.dram_tensor("w1", [2 * F, H], _f32r, kind="ExternalInput")
    b1 = nc.dram_tensor("b1", [H, 1], _f32, kind="ExternalInput")
    w2 = nc.dram_tensor("w2", [H, 1], _f32r, kind="ExternalInput")
    b2 = nc.dram_tensor("b2", [1, 1], _f32, kind="ExternalInput")
    out = nc.dram_tensor("out", [1, EPC], _f32, kind="ExternalOutput")

    ro_r = ro.rearrange("(j p) e -> j p e", p=P)
    ri_r = ri.rearrange("(j p) e -> j p e", p=P)

    with tile.TileContext(nc) as tc:
        with (
            tc.tile_pool(name="consts", bufs=1) as consts,
            tc.tile_pool(name="ro_pool", bufs=2) as ro_pool,
            tc.tile_pool(name="ri_pool", bufs=2) as ri_pool,
            tc.tile_pool(name="mid", bufs=1) as mid,
            tc.tile_pool(name="ps", bufs=8, space="PSUM") as ps,
        ):
            x_sb = consts.tile([P, NCH * F], _f32r)
            nc.gpsimd.dma_start(out=x_sb, in_=x.ap())
            b1_sb = consts.tile([H, 1], _f32)
            nc.gpsimd.dma_start(out=b1_sb, in_=b1.ap())
            w2_sb = consts.tile([H, 1], _f32r)
            nc.gpsimd.dma_start(out=w2_sb, in_=w2.ap())
            b2_sb = consts.tile([P, 1], _f32)
            nc.gpsimd.dma_start(out=b2_sb, in_=b2.ap().to_broadcast([P, 1]))

            acc = {}
            for m in range(2):
                for cb in range(NCB):
                    acc[(m, cb)] = ps.tile([F, NB], _f32, tag="bank", name=f"acc_{m}_{cb}")

            for j0, cq in GROUPS:
                tail = "_tail" if cq < CQ else ""
                ro_t = ro_pool.tile([P, cq, EPC], _f32r, tag="ro" + tail, name=f"ro_t_{j0}")
                nc.sync.dma_start(
                    out=ro_t,
                    in_=ro_r[j0 : j0 + cq].rearrange("c p e -> p c e"),
                )
                ri_t = ri_pool.tile([P, cq, EPC], _f32r, tag="ri" + tail, name=f"ri_t_{j0}")
                nc.scalar.dma_start(
                    out=ri_t,
                    in_=ri_r[j0 : j0 + cq].rearrange("c p e -> p c e"),
                )
                for c in range(cq):
                    j = j0 + c
                    lhsT = x_sb[:, bass.ts(j, F)]  # [128, 16]
                    first = j == 0
                    last = j == NCH - 1
                    for cb in range(NCB):
                        nc.tensor.matmul(
                            acc[(0, cb)],
                            lhsT,
                            ro_t[:, c, bass.ts(cb, NB)],
                            start=first,
                            stop=last,
                        )
                        nc.tensor.matmul(
                            acc[(1, cb)],
                            lhsT,
                            ri_t[:, c, bass.ts(cb, NB)],
                            start=first,
                            stop=last,
                        )

            bo_sb = mid.tile([F, EPC], _f32r)
            bi_sb = mid.tile([F, EPC], _f32r)
            for cb in range(NCB):
                nc.vector.tensor_copy(
                    bo_sb[:, bass.ts(cb, NB)], acc[(0, cb)][:]
                )
                nc.vector.tensor_copy(
                    bi_sb[:, bass.ts(cb, NB)], acc[(1, cb)][:]
                )

            w1a_sb = consts.tile([F, H], _f32r)
            nc.gpsimd.dma_start(out=w1a_sb, in_=w1.ap()[0:F, :])
            w1b_sb = consts.tile([F, H], _f32r)
            nc.gpsimd.dma_start(out=w1b_sb, in_=w1.ap()[F : 2 * F, :])

            h_sb = mid.tile([H, EPC], _f32r)
            out_sb = mid.tile([1, EPC], _f32)
            hps = []
            for cb in range(NCB):
                hp = ps.tile([H, NB], _f32, tag="bank", name=f"hp_{cb}")
                nc.tensor.matmul(
                    hp[:],
                    w1a_sb[:],
                    bo_sb[:, bass.ts(cb, NB)],
                    start=True,
                    stop=False,
                )
                nc.tensor.matmul(
                    hp[:],
                    w1b_sb[:],
                    bi_sb[:, bass.ts(cb, NB)],
                    start=False,
                    stop=True,
                )
                hps.append(hp)
            for cb in range(NCB):
                nc.scalar.activation(
                    h_sb[:, bass.ts(cb, NB)], hps[cb][:], _AF.Tanh, bias=b1_sb[:]
                )
            lps = []
            for cb in range(NCB):
                lp = ps.tile([1, NB], _f32, tag="bank", name=f"lp_{cb}")
                nc.tensor.matmul(
                    lp[:],
                    w2_sb[:],
                    h_sb[:, bass.ts(cb, NB)],
                    start=True,
                    stop=True,
                )
                lps.append(lp)
            for cb in range(NCB):
                nc.scalar.activation(
                    out_sb[:, bass.ts(cb, NB)],
                    lps[cb][:],
                    _AF.Sigmoid,
                    bias=b2_sb[0:1, :],
                )
            nc.sync.dma_start(out=out.ap(), in_=out_sb)

    return nc


# ---------------------------------------------------------------------------
# IR post-processing (shared)
# ---------------------------------------------------------------------------

def _split_multi_waits(nc: bass.Bass, limit: int = 1) -> None:
    """The walrus build in this image lowers at most one sync-wait per
    instruction ("Too many sync wait commands").  Move surplus waits onto
    standalone event-semaphore instructions inserted just before the
    over-subscribed instruction on the same engine — identical sync
    semantics, one wait per instruction."""
    n = 0
    for f in nc.m.functions:
        for bb in f.blocks:
            insts = bb.instructions  # live list
            new_list = []
            for inst in list(insts):
                si = inst.sync_info
                if si is not None and len(si.on_wait) > limit:
                    waits = list(si.on_wait)
                    extra, keep = waits[:-limit], waits[-limit:]
                    for w in extra:
                        n += 1
                        ev = mybir.InstEventSemaphore(
                            name=f"splitwait_{n}", ins=[], outs=[],
                            engine=inst.engine,
                        )
                        ev.sync_info = mybir.SyncInfo(on_wait=[w], on_update=[])
                        nc.register_instruction(ev, overwrite=True)
                        new_list.append(ev)
                    si.on_wait = keep
                new_list.append(inst)
            insts[:] = new_list


def _hoist_lib_load(nc: bass.Bass) -> None:
    """Move the gpsimd library-load (MPC LOAD_LIB) from the kernel block
    into the prologue, ahead of the engine-boot barrier: the ~6-10 us Q7
    iram reload then overlaps engine boot + the idx DMAs instead of
    serializing before the first gather."""
    blocks = nc.m.functions[0].blocks
    if len(blocks) < 2:
        return
    bb0, bb1 = blocks[0], blocks[1]
    keep = []
    hoisted = []
    for inst in list(bb1.instructions):
        if (
            not hoisted
            and type(inst).__name__ == "InstPseudoReloadLibraryIndex"
            and (inst.sync_info is None or not inst.sync_info.on_wait)
        ):
            hoisted.append(inst)
        else:
            keep.append(inst)
    if not hoisted:
        return
    bb1.instructions[:] = keep
    b0 = bb0.instructions
    pos = next(
        (i for i, inst in enumerate(b0) if type(inst).__name__ == "InstDrain"),
        len(b0),
    )
    b0[:] = b0[:pos] + hoisted + b0[pos:]


def _hoist_first_dmas(
    nc: bass.Bass,
    engines=("EngineType.SP", "EngineType.Activation"),
    n_per_engine: int = 1,
) -> None:
    """Move the first no-wait DMAs per listed engine (consumers gate on
    their completion semaphores) from the kernel block into the prologue
    block, ahead of the ~4 us engine-boot barrier, so the HBM stream
    starts as soon as the sequencers are up."""
    blocks = nc.m.functions[0].blocks
    if len(blocks) < 2:
        return
    bb0, bb1 = blocks[0], blocks[1]
    hoisted = []
    seen = {}
    b1_insts = bb1.instructions
    keep = []
    for inst in list(b1_insts):
        eng = str(inst.engine)
        if (
            type(inst).__name__ == "InstDMACopy"
            and eng in engines
            and seen.get(eng, 0) < n_per_engine
            and (inst.sync_info is None or not inst.sync_info.on_wait)
        ):
            seen[eng] = seen.get(eng, 0) + 1
            hoisted.append(inst)
        else:
            keep.append(inst)
    if not hoisted:
        return
    b1_insts[:] = keep
    b0 = bb0.instructions
    pos = next(
        (i for i, inst in enumerate(b0) if type(inst).__name__ == "InstDrain"),
        len(b0),
    )
    b0[:] = b0[:pos] + hoisted + b0[pos:]


_NC_CACHE = {}

def _get_nc(which: str) -> bass.Bass:
    if which not in _NC_CACHE:
        if which == "apg":
            nc = _build_nc_apg()
            # Raw Bass skips the extended-inst codegen pass; without it the
            # NEFF compiler sees empty .instr -> "ISA wrong length".
            mybir.codegen_inst_isa_subclasses(nc)
            _hoist_lib_load(nc)
            _hoist_first_dmas(nc, n_per_engine=2)
        else:
            nc = _build_nc_dense()
            _hoist_first_dmas(nc)
        _split_multi_waits(nc)
        _NC_CACHE[which] = nc
    return _NC_CACHE[which]


# ---------------------------------------------------------------------------
# Host-side input marshalling
# ---------------------------------------------------------------------------

def _extract_indices(Rmat: np.ndarray):
    """Losslessly re-encode one-hot [B, N, E] as int indices [B, E].
    Returns None unless every column is verified (exactly, via probe
    GEMMs) to contain a single 1.0 and zeros elsewhere."""
    Rmat = np.asarray(Rmat)
    if Rmat.dtype != np.float32 or Rmat.shape != (B, N, E):
        return None
    probe = np.empty((3, N), np.float32)
    probe[0] = 1.0
    probe[1] = np.arange(N, dtype=np.float32)
    rng = np.random.default_rng(0xEDE0)
    probe[2] = rng.uniform(1.0, 2.0, N).astype(np.float32)
    idx = np.empty((B, E), np.int32)
    for b in range(B):
        V = probe @ Rmat[b]  # [3, E], one pass over the 128 MB matrix
        if not np.all(V[0] == 1.0):
            return None
        ir = np.rint(V[1])
        if not np.array_equal(V[1], ir):
            return None
        i64 = ir.astype(np.int64)
        if i64.min() < 0 or i64.max() >= N:
            return None
        # single-support columns reproduce the probe value bit-exactly
        if not np.array_equal(V[2], probe[2][i64]):
            return None
        idx[b] = i64
    return idx


def _wrap_idx_apg(o_slice: np.ndarray, i_slice: np.ndarray) -> np.ndarray:
    """ap_gather index layout: slot q's 512 Ro idxs wrapped over the even
    Q7 core's 16 partitions (32q..32q+15), its 512 Ri idxs over the odd
    core's (32q+16..32q+31).  Position j sits at partition j%16, column
    j//16, so a column range [8k, 8k+8) is exactly the idx-position
    range [128k, 128k+128) of every core -- the chunked gathers slice
    columns without re-wrapping."""
    w = np.empty((P, GN // 16), np.int16)
    for q in range(GQ):
        w[32 * q : 32 * q + 16] = (
            o_slice[q * GN : (q + 1) * GN].reshape(GN // 16, 16).T
        )
        w[32 * q + 16 : 32 * q + 32] = (
            i_slice[q * GN : (q + 1) * GN].reshape(GN // 16, 16).T
        )
    return np.ascontiguousarray(w)


def _make_in_maps_apg(X, Ri, Ro, W1, b1, W2, b2, idx_i, idx_o):
    X = np.asarray(X, np.float32)
    W1 = np.asarray(W1, np.float32)
    b1 = np.asarray(b1, np.float32)
    W2 = np.asarray(W2, np.float32)
    b2 = np.asarray(b2, np.float32)

    # X^T replicated into all eight 16-partition Q7 slices, shipped bf16
    # (1 MB) and upcast on-chip to the f32 gather table.
    xtbs = []
    for b in range(B):
        xtbs.append(np.ascontiguousarray(np.tile(X[b].T.astype(_BF), (P // F, 1))))
    w1u = W1.astype(_BF).view(np.uint16)
    w2u = W2.reshape(H, 1).astype(_BF).view(np.uint16)
    b1u = b1.reshape(H, 1).view(np.uint16)  # f32 -> 2 uint16 cols
    b2u = b2.reshape(1, 1).view(np.uint16)

    in_maps = []
    for core in range(NCORES):
        b = core // SPB
        e0 = (core % SPB) * EPC
        auxc = np.zeros((P, AUXW), np.uint16)
        auxc[:, 0:32] = _wrap_idx_apg(
            idx_o[b, e0 : e0 + EPC], idx_i[b, e0 : e0 + EPC]
        ).view(np.uint16)
        auxc[0 : 2 * F, 32:96] = w1u
        auxc[0:H, 96:98] = b1u
        auxc[0:H, 98:99] = w2u
        auxc[0:1, 100:102] = b2u
        in_maps.append({"xtb": xtbs[b], "aux": auxc.view(_BF)})
    return in_maps


def _make_in_maps_dense(X, Ri, Ro, W1, b1, W2, b2):
    X = np.asarray(X, np.float32)
    W1 = np.asarray(W1, np.float32)
    b1 = np.asarray(b1, np.float32)
    W2 = np.asarray(W2, np.float32)
    b2 = np.asarray(b2, np.float32)
    in_maps = []
    for core in range(NCORES):
        b = core // SPB
        e0 = (core % SPB) * EPC
        xp = np.ascontiguousarray(
            X[b].reshape(NCH, P, F).transpose(1, 0, 2).reshape(P, NCH * F)
        )
        in_maps.append(
            {
                "x": xp,
                "ro": np.ascontiguousarray(np.asarray(Ro)[b, :, e0 : e0 + EPC]),
                "ri": np.ascontiguousarray(np.asarray(Ri)[b, :, e0 : e0 + EPC]),
                "w1": W1,
                "b1": b1.reshape(H, 1),
                "w2": W2.reshape(H, 1),
                "b2": b2.reshape(1, 1),
            }
        )
    return in_maps


def run(inputs: dict, trace: bool = False, trace_cores=None, force_path=None):
    """Run the kernel; returns (full_output, BassKernelResults)."""
    path = force_path
    idx_i = idx_o = None
    if path is None or path == "apg":
        idx_i = _extract_indices(inputs["Ri"])
        idx_o = _extract_indices(inputs["Ro"]) if idx_i is not None else None
        if path is None:
            path = "apg" if idx_o is not None else "dense"
        elif idx_o is None:
            raise ValueError("index path forced but inputs are not one-hot")

    nc = _get_nc(path)
    if path == "apg":
        in_maps = _make_in_maps_apg(idx_i=idx_i, idx_o=idx_o, **inputs)
    else:
        in_maps = _make_in_maps_dense(**inputs)
    bkr = run_bass_kernel_spmd(
        nc,
        in_maps,
        core_ids=list(range(NCORES)),
        trace=trace,
        trace_cores=trace_cores,
    )
    out = np.empty((B, E), np.float32)
    for core in range(NCORES):
        b = core // SPB
        e0 = (core % SPB) * EPC
        res = bkr.results[core]["out"]
        if path == "apg":
            # Row r covers flat columns [s, s+w) of chunk k's [32, 4*cw]
            # feat block; flat col c maps to slot q = c // cw, position
            # j = c % cw, i.e. edge 512q + COFF[k] + j.
            res = res.reshape(4 * OUTW)[_EMAP_SRC]
        else:
            res = res.reshape(EPC)
        out[b, e0 : e0 + EPC] = res
    return out, bkr


def kernel(**inputs) -> np.ndarray:
    out, _ = run(inputs)
    return out


# revision 32
# speedup vs baseline: 1.0448x; 1.0448x over previous
"""EdgeNetwork GNN message-passing kernel for 8 Trainium2 NeuronCores.

Math (per batch b):
    bo = Ro[b]^T @ X[b]            # [E, F]  gather of outgoing-node feats
    bi = Ri[b]^T @ X[b]            # [E, F]
    feats = concat(bo, bi)         # [E, 2F]
    h = tanh(feats @ W1 + b1)      # [E, H]
    out = sigmoid(h @ W2 + b2)     # [E]

Sharding: 8 shards over (B=2) x (E/4): each core owns one (b, 2048-edge)
slice with zero cross-core communication.

Fast path: Ri/Ro produced by jax.nn.one_hot are exactly one-hot, so the
einsum is a row gather X[idx].  The host losslessly re-encodes each
one-hot matrix as its int16 index vector (and verifies exactness with
probe GEMVs -- any non-one-hot input falls back to the dense kernel).
On device an ap_gather over a replicated X^T table delivers the
gathered features already transposed (features-on-partitions), feeding
the MLP directly.

This revision vs the 43 us baseline:
  * X^T ships as bf16 [128, 4096] (1 MB, host-replicated) and is
    upcast on-chip to the f32 gather table split across DVE + Act,
    instead of DMAing the 2 MB f32 table (which took ~10 us and held
    the gather until ~19.7 us).
  * The single 512-idx ap_gather (14 us of Q7 time, the dominant cost:
    ~110 ns per 4-idx request, invariant to layout) is split into 4
    column-range chunks of 128 idxs; all 8 Q7 cores stay busy in each
    chunk and the MLP for chunk k runs under the gather of chunk k+1.
  * The MLP runs in bf16: one K=32 layer-1 matmul per chunk (the DVE
    rebase-casts double as the f32 -> bf16 convert), tanh -> bf16,
    K=64 layer-2, sigmoid into partitions 0/32/64/96, one output DMA.
    Host reorders the chunk-major output (pure index bookkeeping).

Dense fallback: streams the full one-hot matrices through the
TensorEngine as float32r matmuls (HBM-bound, 64 MB/core).
"""

import numpy as np
import ml_dtypes

import concourse.bass as bass
import concourse.mybir as mybir
import concourse.tile as tile
from concourse import library_config
from concourse.bass_utils import run_bass_kernel_spmd

B, N, E, F, H = 2, 4096, 8192, 16, 64
NCORES = 8
SPB = NCORES // B          # edge shards per batch = 4
EPC = E // SPB             # edges per core = 2048
P = 128                    # partition size
NCH = N // P               # 32 node chunks
CQ = 3                     # node chunks per big DMA transfer (dense path)
GROUPS = [(g * CQ, CQ) for g in range(10)] + [(30, 1), (31, 1)]
NB = 512                   # PSUM bank width in f32
NCB = EPC // NB            # 4 column blocks per core

_f32 = mybir.dt.float32
_f32r = mybir.dt.float32r
_bf16 = mybir.dt.bfloat16
_i16 = mybir.dt.int16
_AF = mybir.ActivationFunctionType
_BF = ml_dtypes.bfloat16

# ap_gather layout: 4 edge groups (slots) of 512, each owning a
# 32-partition slice: even Q7 core = 16 Ro-gather feature channels,
# odd Q7 core = 16 Ri-gather channels, so each slot holds the
# [bo; bi] feats concat and layer 1 is a single K=32 matmul.
GQ = 4
GN = EPC // GQ             # 512 idxs per Q7 core (= slot edge count)

# The gather is chunked along the idx dimension so the MLP pipelines
# under it (all 8 Q7 cores stay busy in every chunk).  Each ap_gather
# instruction costs ~1.3 us beyond its ~96 ns/4-idx request stream, so
# two chunks beat four (measured: 4x128 = 4.7 us each); the second
# chunk is smaller so the un-hidden tail MLP after the last gather is
# shorter, while chunk 1's larger MLP still fits under chunk 2's
# gather shadow.
CWS = [256, 256]           # idxs per core per chunk (sum = GN = 512)
COFF = [0, 256]
NCHUNK = len(CWS)
# 512-col half-splits of each chunk's 4*CW feat columns (PSUM banks).
CHALVES = [[(0, 512), (512, 512)], [(0, 512), (512, 512)]]
NROWS = sum(len(h) for h in CHALVES)  # output rows (one per half)
# Activation outputs may only start at partitions 0/32/64/96:
# (partition slot, col) per output row.
RPLACE = [(0, 0), (1, 0), (2, 0), (3, 0)]
OUTW = NB
# The 1 MB bf16 table ships as two sequential column-half DMAs on the
# sync queue so half 1 upcasts under half 2's transfer.  Half 2's
# upcast is split between DVE (~0.69 ns/col) and Act (~1.28 ns/col).
XH = N // 2
UPC_DVE = XH + 1408
AUXW = 104                 # packed idx+weights tensor width (bf16 cols)


def _build_emap():
    """_EMAP_SRC[e] = flat index into the device's [4, OUTW] output
    holding edge e's sigmoid (e relative to the core's 2048-edge slice)."""
    src = np.empty(EPC, np.int64)
    row = 0
    for k in range(NCHUNK):
        cw = CWS[k]
        for s, w in CHALVES[k]:
            slot, col = RPLACE[row]
            c = s + np.arange(w)
            edge = 512 * (c // cw) + COFF[k] + (c % cw)
            src[edge] = slot * OUTW + col + np.arange(w)
            row += 1
    return src


for _k in range(NCHUNK):
    assert sum(w for _, w in CHALVES[_k]) == GQ * CWS[_k]
_EMAP_SRC = _build_emap()


# ---------------------------------------------------------------------------
# ap_gather device kernel: on-chip free-dim gather over X^T
# ---------------------------------------------------------------------------

def _build_nc_apg() -> bass.Bass:
    nc = bass.Bass()

    xtb = nc.dram_tensor("xtb", [P, N], _bf16, kind="ExternalInput")
    # idx + W1 + b1 + W2 + b2 packed into one raw-bytes tensor: every DMA
    # instruction costs ~0.65 us of sequencer issue time, so one DMA
    # replaces five.  Layout (bf16 columns): [0:32) idx int16, [32:96)
    # W1 bf16 on partitions 0-31, [96:98) b1 f32 on partitions 0-63,
    # [98:99) W2 bf16 on partitions 0-63, [100:102) b2 f32 on partition 0.
    aux = nc.dram_tensor("aux", [P, AUXW], _bf16, kind="ExternalInput")
    out = nc.dram_tensor("out", [4, OUTW], _f32, kind="ExternalOutput")

    with tile.TileContext(nc) as tc:
        with (
            tc.tile_pool(name="consts", bufs=1) as consts,
            tc.tile_pool(name="mid", bufs=1) as mid,
            tc.tile_pool(name="ps", bufs=8, space="PSUM") as ps,
        ):
            nc.gpsimd.load_library(library_config.ap_gather)

            # Table halves go back-to-back on the sync queue (its rings
            # process them FIFO, so half 1 lands ~2 us before half 2) as
            # SEPARATE tiles -- a split write into one tile picks up
            # framework waits that defeat the pre-boot hoist.  The packed
            # aux blob rides the scalar queue.
            xtb1_sb = mid.tile([P, XH], _bf16)
            nc.sync.dma_start(out=xtb1_sb, in_=xtb.ap()[:, 0:XH])
            xtb2_sb = mid.tile([P, N - XH], _bf16)
            nc.sync.dma_start(out=xtb2_sb, in_=xtb.ap()[:, XH:N])
            aux_sb = consts.tile([P, AUXW], _bf16)
            nc.scalar.dma_start(out=aux_sb, in_=aux.ap())

            idx_sb = aux_sb[:, 0:32]
            w1_sb = aux_sb[0 : 2 * F, 32:96]
            b1_sb = aux_sb[0:H, 96:98].bitcast(_f32)
            w2_sb = aux_sb[0:H, 98:99]
            b2_sb = aux_sb[0:1, 100:102].bitcast(_f32)

            # Preload the activation LUT set on the Act engine before its
            # upcast share (the first activation otherwise pays the ~1.3 us
            # table load on the critical path).
            warm = consts.tile([H, 1], _f32)
            nc.scalar.activation(warm[:], b1_sb, _AF.Tanh, bias=b1_sb)
            nc.scalar.activation(warm[:], b1_sb, _AF.Sigmoid, bias=b1_sb)

            # NB: the gather table must be plain f32 -- a float32r ISA dtype
            # on the ap_gather instruction wedges the Q7 (measured), and
            # bf16 is illegal for d=1 (elem word must be 4-byte-aligned).
            # DVE upcasts half 1 while half 2 is still in flight; half 2
            # is then split DVE/Act so both finish together.
            xt_sb = mid.tile([P, N, 1], _f32)
            nc.vector.tensor_copy(xt_sb[:, 0:XH, 0], xtb1_sb[:])
            nc.vector.tensor_copy(
                xt_sb[:, XH:UPC_DVE, 0], xtb2_sb[:, 0 : UPC_DVE - XH]
            )
            nc.scalar.copy(xt_sb[:, UPC_DVE:N, 0], xtb2_sb[:, UPC_DVE - XH :])

            dsts = []
            for k in range(NCHUNK):
                dst = mid.tile([P, CWS[k], 1], _f32, name=f"dst_{k}")
                nc.gpsimd.ap_gather(
                    out_ap=dst[:],
                    in_ap=xt_sb[:],
                    idxs_ap=idx_sb[
                        :, COFF[k] // 16 : (COFF[k] + CWS[k]) // 16
                    ].bitcast(_i16),
                    channels=P,
                    num_elems=N,
                    d=1,
                    num_idxs=CWS[k],
                )
                dsts.append(dst)

            out_sb = mid.tile([P, OUTW], _f32)
            row = 0
            for k in range(NCHUNK):
                cw = CWS[k]
                # Rebase each 32-partition slot (= [bo; bi] feats block) to
                # partition 0 for the PE while converting f32 -> bf16.
                fc = mid.tile([2 * F, GQ * cw], _bf16, name=f"fc_{k}")
                for q in range(GQ):
                    nc.vector.tensor_copy(
                        fc[:, q * cw : (q + 1) * cw],
                        dsts[k][32 * q : 32 * q + 2 * F, :, 0],
                    )
                fcf = fc[:]
                # The chunk's 4*cw columns run as <=512-col slices (one PSUM
                # bank each).  Issue stage-by-stage across the slices so the
                # PE never idles waiting for a tanh of the slice it just
                # produced.
                halves = CHALVES[k]
                hps, hsbs, lps = [], [], []
                for i, (s, w) in enumerate(halves):
                    hp = ps.tile([H, w], _f32, tag="bank", name=f"hp_{row+i}")
                    nc.tensor.matmul(
                        hp[:], w1_sb, fcf[:, s : s + w], start=True, stop=True
                    )
                    hps.append(hp)
                for i, (s, w) in enumerate(halves):
                    h_sb = mid.tile([H, w], _bf16, name=f"h_{row+i}")
                    nc.scalar.activation(
                        h_sb[:], hps[i][:], _AF.Tanh, bias=b1_sb
                    )
                    hsbs.append(h_sb)
                for i, (s, w) in enumerate(halves):
                    lp = ps.tile([1, w], _f32, tag="bank", name=f"lp_{row+i}")
                    nc.tensor.matmul(
                        lp[:], w2_sb, hsbs[i][:], start=True, stop=True
                    )
                    lps.append(lp)
                for i, (s, w) in enumerate(halves):
                    # Output rows pack into partitions 0/32/64/96 (the only
                    # legal Activation output bases) x 1024 cols, so the
                    # final DMA is one multi-partition transfer.
                    slot, col = RPLACE[row + i]
                    nc.scalar.activation(
                        out_sb[32 * slot : 32 * slot + 1, col : col + w],
                        lps[i][:],
                        _AF.Sigmoid,
                        bias=b2_sb,
                    )
                row += len(halves)
            nc.sync.dma_start(out=out.ap(), in_=out_sb[0:P:32, :])

    return nc


# ---------------------------------------------------------------------------
# Dense fallback device kernel (streams full one-hot matrices)
# ---------------------------------------------------------------------------

def _build_nc_dense() -> bass.Bass:
    nc = bass.Bass()

    x = nc.dram_tensor("x", [P, NCH * F], _f32r, kind="ExternalInput")
    ro = nc.dram_tensor("ro", [N, EPC], _f32r, kind="ExternalInput")
    ri = nc.dram_tensor("ri", [N, EPC], _f32r, kind="ExternalInput")
    w1 = nc.dram_tensor("w1", [2 * F, H], _f32r, kind="ExternalInput")
    b1 = nc.dram_tensor("b1", [H, 1], _f32, kind="ExternalInput")
    w2 = nc.dram_tensor("w2", [H, 1], _f32r, kind="ExternalInput")
    b2 = nc.dram_tensor("b2", [1, 1], _f32, kind="ExternalInput")
    out = nc.dram_tensor("out", [1, EPC], _f32, kind="ExternalOutput")

    ro_r = ro.rearrange("(j p) e -> j p e", p=P)
    ri_r = ri.rearrange("(j p) e -> j p e", p=P)

    with tile.TileContext(nc) as tc:
        with (
            tc.tile_pool(name="consts", bufs=1) as consts,
            tc.tile_pool(name="ro_pool", bufs=2) as ro_pool,
            tc.tile_pool(name="ri_pool", bufs=2) as ri_pool,
            tc.tile_pool(name="mid", bufs=1) as mid,
            tc.tile_pool(name="ps", bufs=8, space="PSUM") as ps,
        ):
            x_sb = consts.tile([P, NCH * F], _f32r)
            nc.gpsimd.dma_start(out=x_sb, in_=x.ap())
            b1_sb = consts.tile([H, 1], _f32)
            nc.gpsimd.dma_start(out=b1_sb, in_=b1.ap())
            w2_sb = consts.tile([H, 1], _f32r)
            nc.gpsimd.dma_start(out=w2_sb, in_=w2.ap())
            b2_sb = consts.tile([P, 1], _f32)
            nc.gpsimd.dma_start(out=b2_sb, in_=b2.ap().to_broadcast([P, 1]))

            acc = {}
            for m in range(2):
                for cb in range(NCB):
                    acc[(m, cb)] = ps.tile([F, NB], _f32, tag="bank", name=f"acc_{m}_{cb}")

            for j0, cq in GROUPS:
                tail = "_tail" if cq < CQ else ""
                ro_t = ro_pool.tile([P, cq, EPC], _f32r, tag="ro" + tail, name=f"ro_t_{j0}")
                nc.sync.dma_start(
                    out=ro_t,
                    in_=ro_r[j0 : j0 + cq].rearrange("c p e -> p c e"),
                )
                ri_t = ri_pool.tile([P, cq, EPC], _f32r, tag="ri" + tail, name=f"ri_t_{j0}")
                nc.scalar.dma_start(
                    out=ri_t,
                    in_=ri_r[j0 : j0 + cq].rearrange("c p e -> p c e"),
                )
                for c in range(cq):
                    j = j0 + c
                    lhsT = x_sb[:, bass.ts(j, F)]  # [128, 16]
                    first = j == 0
                    last = j == NCH - 1
                    for cb in range(NCB):
                        nc.tensor.matmul(
                            acc[(0, cb)],
                            lhsT,
                            ro_t[:, c, bass.ts(cb, NB)],
                            start=first,
                            stop=last,
                        )
                        nc.tensor.matmul(
                            acc[(1, cb)],
                            lhsT,
                            ri_t[:, c, bass.ts(cb, NB)],
                            start=first,
                            stop=last,
                        )

            bo_sb = mid.tile([F, EPC], _f32r)
            bi_sb = mid.tile([F, EPC], _f32r)
            for cb in range(NCB):
                nc.vector.tensor_copy(
                    bo_sb[:, bass.ts(cb, NB)], acc[(0, cb)][:]
                )
                nc.vector.tensor_copy(
                    bi_sb[:, bass.ts(cb, NB)], acc[(1, cb)][:]
                )

            w1a_sb = consts.tile([F, H], _f32r)
            nc.gpsimd.dma_start(out=w1a_sb, in_=w1.ap()[0:F, :])
            w1b_sb = consts.tile([F, H], _f32r)
            nc.gpsimd.dma_start(out=w1b_sb, in_=w1.ap()[F : 2 * F, :])

            h_sb = mid.tile([H, EPC], _f32r)
            out_sb = mid.tile([1, EPC], _f32)
            hps = []
            for cb in range(NCB):
                hp = ps.tile([H, NB], _f32, tag="bank", name=f"hp_{cb}")
                nc.tensor.matmul(
                    hp[:],
                    w1a_sb[:],
                    bo_sb[:, bass.ts(cb, NB)],
                    start=True,
                    stop=False,
                )
                nc.tensor.matmul(
                    hp[:],
                    w1b_sb[:],
                    bi_sb[:, bass.ts(cb, NB)],
                    start=False,
                    stop=True,
                )
                hps.append(hp)
            for cb in range(NCB):
                nc.scalar.activation(
                    h_sb[:, bass.ts(cb, NB)], hps[cb][:], _AF.Tanh, bias=b1_sb[:]
                )
            lps = []
            for cb in range(NCB):
                lp = ps.tile([1, NB], _f32, tag="bank", name=f"lp_{cb}")
                nc.tensor.matmul(
                    lp[:],
                    w2_sb[:],
                    h_sb[:, bass.ts(cb, NB)],
                    start=True,
                    stop=True,
                )
                lps.append(lp)
            for cb in range(NCB):
                nc.scalar.activation(
                    out_sb[:, bass.ts(cb, NB)],
                    lps[cb][:],
                    _AF.Sigmoid,
                    bias=b2_sb[0:1, :],
                )
            nc.sync.dma_start(out=out.ap(), in_=out_sb)

    return nc


# ---------------------------------------------------------------------------
# IR post-processing (shared)
# ---------------------------------------------------------------------------

def _split_multi_waits(nc: bass.Bass, limit: int = 1) -> None:
    """The walrus build in this image lowers at most one sync-wait per
    instruction ("Too many sync wait commands").  Move surplus waits onto
    standalone event-semaphore instructions inserted just before the
    over-subscribed instruction on the same engine — identical sync
    semantics, one wait per instruction."""
    n = 0
    for f in nc.m.functions:
        for bb in f.blocks:
            insts = bb.instructions  # live list
            new_list = []
            for inst in list(insts):
                si = inst.sync_info
                if si is not None and len(si.on_wait) > limit:
                    waits = list(si.on_wait)
                    extra, keep = waits[:-limit], waits[-limit:]
                    for w in extra:
                        n += 1
                        ev = mybir.InstEventSemaphore(
                            name=f"splitwait_{n}", ins=[], outs=[],
                            engine=inst.engine,
                        )
                        ev.sync_info = mybir.SyncInfo(on_wait=[w], on_update=[])
                        nc.register_instruction(ev, overwrite=True)
                        new_list.append(ev)
                    si.on_wait = keep
                new_list.append(inst)
            insts[:] = new_list


def _hoist_lib_load(nc: bass.Bass) -> None:
    """Move the gpsimd library-load (MPC LOAD_LIB) from the kernel block
    into the prologue, ahead of the engine-boot barrier: the ~6-10 us Q7
    iram reload then overlaps engine boot + the idx DMAs instead of
    serializing before the first gather."""
    blocks = nc.m.functions[0].blocks
    if len(blocks) < 2:
        return
    bb0, bb1 = blocks[0], blocks[1]
    keep = []
    hoisted = []
    for inst in list(bb1.instructions):
        if (
            not hoisted
            and type(inst).__name__ == "InstPseudoReloadLibraryIndex"
            and (inst.sync_info is None or not inst.sync_info.on_wait)
        ):
            hoisted.append(inst)
        else:
            keep.append(inst)
    if not hoisted:
        return
    bb1.instructions[:] = keep
    b0 = bb0.instructions
    pos = next(
        (i for i, inst in enumerate(b0) if type(inst).__name__ == "InstDrain"),
        len(b0),
    )
    b0[:] = b0[:pos] + hoisted + b0[pos:]


def _hoist_first_dmas(
    nc: bass.Bass,
    engines=("EngineType.SP", "EngineType.Activation"),
    n_per_engine: int = 1,
) -> None:
    """Move the first no-wait DMAs per listed engine (consumers gate on
    their completion semaphores) from the kernel block into the prologue
    block, ahead of the ~4 us engine-boot barrier, so the HBM stream
    starts as soon as the sequencers are up."""
    blocks = nc.m.functions[0].blocks
    if len(blocks) < 2:
        return
    bb0, bb1 = blocks[0], blocks[1]
    hoisted = []
    seen = {}
    b1_insts = bb1.instructions
    keep = []
    for inst in list(b1_insts):
        eng = str(inst.engine)
        if (
            type(inst).__name__ == "InstDMACopy"
            and eng in engines
            and seen.get(eng, 0) < n_per_engine
            and (inst.sync_info is None or not inst.sync_info.on_wait)
        ):
            seen[eng] = seen.get(eng, 0) + 1
            hoisted.append(inst)
        else:
            keep.append(inst)
    if not hoisted:
        return
    b1_insts[:] = keep
    b0 = bb0.instructions
    pos = next(
        (i for i, inst in enumerate(b0) if type(inst).__name__ == "InstDrain"),
        len(b0),
    )
    b0[:] = b0[:pos] + hoisted + b0[pos:]


_NC_CACHE = {}

def _get_nc(which: str) -> bass.Bass:
    if which not in _NC_CACHE:
        if which == "apg":
            nc = _build_nc_apg()
            # Raw Bass skips the extended-inst codegen pass; without it the
            # NEFF compiler sees empty .instr -> "ISA wrong length".
            mybir.codegen_inst_isa_subclasses(nc)
            _hoist_lib_load(nc)
            _hoist_first_dmas(nc, n_per_engine=2)
        else:
            nc = _build_nc_dense()
            _hoist_first_dmas(nc)
        _split_multi_waits(nc)
        _NC_CACHE[which] = nc
    return _NC_CACHE[which]


# ---------------------------------------------------------------------------
# Host-side input marshalling
# ---------------------------------------------------------------------------

def _extract_indices(Rmat: np.ndarray):
    """Losslessly re-encode one-hot [B, N, E] as int indices [B, E].
    Returns None unless every column is verified (exactly, via probe
    GEMMs) to contain a single 1.0 and zeros elsewhere."""
    Rmat = np.asarray(Rmat)
    if Rmat.dtype != np.float32 or Rmat.shape != (B, N, E):
        return None
    probe = np.empty((3, N), np.float32)
    probe[0] = 1.0
    probe[1] = np.arange(N, dtype=np.float32)
    rng = np.random.default_rng(0xEDE0)
    probe[2] = rng.uniform(1.0, 2.0, N).astype(np.float32)
    idx = np.empty((B, E), np.int32)
    for b in range(B):
        V = probe @ Rmat[b]  # [3, E], one pass over the 128 MB matrix
        if not np.all(V[0] == 1.0):
            return None
        ir = np.rint(V[1])
        if not np.array_equal(V[1], ir):
            return None
        i64 = ir.astype(np.int64)
        if i64.min() < 0 or i64.max() >= N:
            return None
        # single-support columns reproduce the probe value bit-exactly
        if not np.array_equal(V[2], probe[2][i64]):
            return None
        idx[b] = i64
    return idx


def _wrap_idx_apg(o_slice: np.ndarray, i_slice: np.ndarray) -> np.ndarray:
    """ap_gather index layout: slot q's 512 Ro idxs wrapped over the even
    Q7 core's 16 partitions (32q..32q+15), its 512 Ri idxs over the odd
    core's (32q+16..32q+31).  Position j sits at partition j%16, column
    j//16, so a column range [8k, 8k+8) is exactly the idx-position
    range [128k, 128k+128) of every core -- the chunked gathers slice
    columns without re-wrapping."""
    w = np.empty((P, GN // 16), np.int16)
    for q in range(GQ):
        w[32 * q : 32 * q + 16] = (
            o_slice[q * GN : (q + 1) * GN].reshape(GN // 16, 16).T
        )
        w[32 * q + 16 : 32 * q + 32] = (
            i_slice[q * GN : (q + 1) * GN].reshape(GN // 16, 16).T
        )
    return np.ascontiguousarray(w)


def _make_in_maps_apg(X, Ri, Ro, W1, b1, W2, b2, idx_i, idx_o):
    X = np.asarray(X, np.float32)
    W1 = np.asarray(W1, np.float32)
    b1 = np.asarray(b1, np.float32)
    W2 = np.asarray(W2, np.float32)
    b2 = np.asarray(b2, np.float32)

    # X^T replicated into all eight 16-partition Q7 slices, shipped bf16
    # (1 MB) and upcast on-chip to the f32 gather table.
    xtbs = []
    for b in range(B):
        xtbs.append(np.ascontiguousarray(np.tile(X[b].T.astype(_BF), (P // F, 1))))
    w1u = W1.astype(_BF).view(np.uint16)
    w2u = W2.reshape(H, 1).astype(_BF).view(np.uint16)
    b1u = b1.reshape(H, 1).view(np.uint16)  # f32 -> 2 uint16 cols
    b2u = b2.reshape(1, 1).view(np.uint16)

    in_maps = []
    for core in range(NCORES):
        b = core // SPB
        e0 = (core % SPB) * EPC
        auxc = np.zeros((P, AUXW), np.uint16)
        auxc[:, 0:32] = _wrap_idx_apg(
            idx_o[b, e0 : e0 + EPC], idx_i[b, e0 : e0 + EPC]
        ).view(np.uint16)
        auxc[0 : 2 * F, 32:96] = w1u
        auxc[0:H, 96:98] = b1u
        auxc[0:H, 98:99] = w2u
        auxc[0:1, 100:102] = b2u
        in_maps.append({"xtb": xtbs[b], "aux": auxc.view(_BF)})
    return in_maps


def _make_in_maps_dense(X, Ri, Ro, W1, b1, W2, b2):
    X = np.asarray(X, np.float32)
    W1 = np.asarray(W1, np.float32)
    b1 = np.asarray(b1, np.float32)
    W2 = np.asarray(W2, np.float32)
    b2 = np.asarray(b2, np.float32)
    in_maps = []
    for core in range(NCORES):
        b = core // SPB
        e0 = (core % SPB) * EPC
        xp = np.ascontiguousarray(
            X[b].reshape(NCH, P, F).transpose(1, 0, 2).reshape(P, NCH * F)
        )
        in_maps.append(
            {
                "x": xp,
                "ro": np.ascontiguousarray(np.asarray(Ro)[b, :, e0 : e0 + EPC]),
                "ri": np.ascontiguousarray(np.asarray(Ri)[b, :, e0 : e0 + EPC]),
                "w1": W1,
                "b1": b1.reshape(H, 1),
                "w2": W2.reshape(H, 1),
                "b2": b2.reshape(1, 1),
            }
        )
    return in_maps


def run(inputs: dict, trace: bool = False, trace_cores=None, force_path=None):
    """Run the kernel; returns (full_output, BassKernelResults)."""
    path = force_path
    idx_i = idx_o = None
    if path is None or path == "apg":
        idx_i = _extract_indices(inputs["Ri"])
        idx_o = _extract_indices(inputs["Ro"]) if idx_i is not None else None
        if path is None:
            path = "apg" if idx_o is not None else "dense"
        elif idx_o is None:
            raise ValueError("index path forced but inputs are not one-hot")

    nc = _get_nc(path)
    if path == "apg":
        in_maps = _make_in_maps_apg(idx_i=idx_i, idx_o=idx_o, **inputs)
    else:
        in_maps = _make_in_maps_dense(**inputs)
    bkr = run_bass_kernel_spmd(
        nc,
        in_maps,
        core_ids=list(range(NCORES)),
        trace=trace,
        trace_cores=trace_cores,
    )
    out = np.empty((B, E), np.float32)
    for core in range(NCORES):
        b = core // SPB
        e0 = (core % SPB) * EPC
        res = bkr.results[core]["out"]
        if path == "apg":
            # Row r covers flat columns [s, s+w) of chunk k's [32, 4*cw]
            # feat block; flat col c maps to slot q = c // cw, position
            # j = c % cw, i.e. edge 512q + COFF[k] + j.
            res = res.reshape(4 * OUTW)[_EMAP_SRC]
        else:
            res = res.reshape(EPC)
        out[b, e0 : e0 + EPC] = res
    return out, bkr


def kernel(**inputs) -> np.ndarray:
    out, _ = run(inputs)
    return out
